# revision 1
# baseline (speedup 1.0000x reference)
"""Trainium2 Bass kernel for nn_BearingQCCFeatureMotion.

Pipeline (B=2, F=8, P=2048, SCALES=(5,15,40)):
  - host (numpy fp32, mirrors the reference formulas): bearing quaternions,
    per-transition relative quaternions q_fwd, squared norms; builds augmented
    matmul tables so the device computes both the pairwise neg-distance matrix
    D[p,q] = 2<x_p,x_q> - |x_p|^2 - |x_q|^2 and the quaternion-dot matrix
    Q[p,q] = <q_fwd[p], q_fwd[q]> as K=5 / K=4 matmuls.
  - device (8 cores, data-parallel over the P dimension; each core handles a
    256-row chunk of all 14 (b,t) transitions):
      PE   : D and Q matmuls into PSUM
      ACT  : PSUM->SBUF moves, |Q|, (1-x), (1+x), sqrt, arctan
      DVE  : exact top-48 per row via 6x max8 + 5x match_replace, masked
             geo-sum for k=40 (scalar_tensor_tensor with accum)
      GPSIMD: clip, divide, masked geo-sums for k=5,15, count at rank-40
    geo is computed as arccos(x) = 2*arctan(sqrt((1-x)/(1+x))), x = |Q| clipped
    to <= 1-1e-7 (the factor 2*2 = 4 is folded into the host post-processing).
  - host: means over transitions, tie/near-tie detection at the rank
    boundaries (fixed by exact fp32 recomputation of the few flagged rows),
    lower-median scale, exp, broadcast to the (B, 3, F*P) output.
"""

import os
import numpy as np

B, F, P = 2, 8, 2048
T = F - 1
NBT = B * T            # 14 independent (b, t) transitions
NCORES = 8
CH = P // NCORES       # 256 rows per core
SCALES = (5, 15, 40)
AUGK = 5               # [2x, 2y, 2z, sq, 1] . [x', y', z', -1, -sq']
QK = 4

_PROG = None


def _build_program(repeat=1):
    """Build + compile the per-core Bass/Tile program (same for all cores)."""
    from contextlib import ExitStack
    import concourse.tile as tile
    from concourse import bacc, mybir
    import concourse.bass as bass

    f32 = mybir.dt.float32
    Alu = mybir.AluOpType
    Act = mybir.ActivationFunctionType

    nc = bacc.Bacc("TRN2", target_bir_lowering=False, debug=False)

    aug_lhsT = nc.dram_tensor("aug_lhsT", [NBT, AUGK, CH], f32,
                              kind="ExternalInput").ap()
    aug_rhs = nc.dram_tensor("aug_rhs", [NBT, AUGK, P], f32,
                             kind="ExternalInput").ap()
    qf_lhsT = nc.dram_tensor("qf_lhsT", [NBT, QK, CH], f32,
                             kind="ExternalInput").ap()
    qf_rhs = nc.dram_tensor("qf_rhs", [NBT, QK, P], f32,
                            kind="ExternalInput").ap()
    sums_o = nc.dram_tensor("sums", [NBT, CH, 4], f32,
                            kind="ExternalOutput").ap()
    maxv_o = nc.dram_tensor("maxv", [NBT, CH, 40], f32,
                            kind="ExternalOutput").ap()

    with tile.TileContext(nc) as tc, ExitStack() as ctx:
        tabs = ctx.enter_context(tc.tile_pool(name="tabs", bufs=2))
        psum_d = ctx.enter_context(tc.tile_pool(name="psd", bufs=1, space="PSUM"))
        psum_q = ctx.enter_context(tc.tile_pool(name="psq", bufs=1, space="PSUM"))
        work = ctx.enter_context(tc.tile_pool(name="work", bufs=2))
        small = ctx.enter_context(tc.tile_pool(name="small", bufs=3))

        const = ctx.enter_context(tc.tile_pool(name="const", bufs=1))
        eps_t = const.tile([128, 1], f32)
        nc.vector.memset(eps_t[:], 1e-38)

        for bt in [i % NBT for i in range(NBT * repeat)]:
            # per-(b,t) tables; matmul operands must start at partition 0
            a_rhs = tabs.tile([AUGK, P], f32, tag="a_rhs")
            nc.sync.dma_start(a_rhs[:], aug_rhs[bt])
            q_rhs = tabs.tile([QK, P], f32, tag="q_rhs")
            nc.sync.dma_start(q_rhs[:], qf_rhs[bt])
            a_lhs = tabs.tile([AUGK, CH], f32, tag="a_lhs")
            nc.sync.dma_start(a_lhs[:], aug_lhsT[bt])
            q_lhs = tabs.tile([QK, CH], f32, tag="q_lhs")
            nc.sync.dma_start(q_lhs[:], qf_lhsT[bt])

            for ch in range(CH // 128):
                ro = ch * 128

                psd = psum_d.tile([128, P], f32)
                for j in range(4):
                    nc.tensor.matmul(
                        psd[:, bass.ts(j, 512)],
                        lhsT=a_lhs[:, ro:ro + 128],
                        rhs=a_rhs[:, bass.ts(j, 512)],
                        start=True, stop=True)
                psq = psum_q.tile([128, P], f32)
                for j in range(4):
                    nc.tensor.matmul(
                        psq[:, bass.ts(j, 512)],
                        lhsT=q_lhs[:, ro:ro + 128],
                        rhs=q_rhs[:, bass.ts(j, 512)],
                        start=True, stop=True)

                # D: pristine copy for masks; the first match_replace below
                # materializes the working copy (out != in), saving a copy
                dsb = work.tile([128, P], f32)
                nc.scalar.copy(dsb[:], psd[:])

                # geo chain: g = arctan(sqrt((1-x)/(1+x))), x = |Q| clipped;
                # 1/(1+x) = sigmoid(-ln(x)) keeps the division off the DVE
                x = work.tile([128, P], f32)
                nc.scalar.activation(x[:], psq[:], Act.Abs)
                num = work.tile([128, P], f32)
                nc.scalar.activation(num[:], x[:], Act.Copy, bias=1.0, scale=-1.0)
                nc.gpsimd.tensor_scalar(out=num[:], in0=num[:], scalar1=1e-7,
                                        scalar2=None, op0=Alu.max)
                s = work.tile([128, P], f32)
                nc.scalar.activation(s[:], x[:], Act.Ln, bias=eps_t[:, 0:1], scale=1.0)
                nc.scalar.activation(s[:], s[:], Act.Sigmoid, scale=-1.0)
                nc.gpsimd.tensor_tensor(out=num[:], in0=num[:], in1=s[:],
                                        op=Alu.mult)
                g = work.tile([128, P], f32)
                nc.scalar.activation(g[:], num[:], Act.Sqrt)
                nc.scalar.activation(g[:], g[:], Act.Arctan)

                # exact top-40 values, descending, via 5x max8 / 4x replace
                maxv = small.tile([128, 40], f32)
                dwk = work.tile([128, P], f32)
                nc.vector.max(maxv[:, 0:8], dsb[:])
                nc.vector.match_replace(dwk[:], maxv[:, 0:8], dsb[:], -1e30)
                for r in range(1, 5):
                    nc.vector.max(maxv[:, 8 * r:8 * r + 8], dwk[:])
                    if r < 4:
                        nc.vector.match_replace(dwk[:], maxv[:, 8 * r:8 * r + 8],
                                                dwk[:], -1e30)

                sums = small.tile([128, 4], f32)
                nc.vector.memset(sums[:], 0.0)
                # k=5 sum off the DVE: GPSIMD mask+mult, ACT accumulates.
                # Its threshold (rank 5) is ready after extraction round 1,
                # so this overlaps the remaining extraction rounds.
                for i, col in ((0, 4), (1, 14)):
                    mk = work.tile([128, P], f32, tag=f"mk{i}")
                    nc.gpsimd.tensor_scalar(out=mk[:], in0=dsb[:],
                                            scalar1=maxv[:, col:col + 1],
                                            scalar2=None, op0=Alu.is_ge)
                    nc.gpsimd.tensor_tensor(out=mk[:], in0=mk[:], in1=g[:],
                                            op=Alu.mult)
                    nc.scalar.activation(mk[:], mk[:], Act.Copy,
                                         accum_out=sums[:, i:i + 1])
                junk_v = work.tile([128, P], f32)
                nc.vector.scalar_tensor_tensor(
                    out=junk_v[:], in0=dsb[:], scalar=maxv[:, 39:40],
                    in1=g[:], op0=Alu.is_ge, op1=Alu.mult,
                    accum_out=sums[:, 2:3])
                # near-tie flag for rank 40: count of D >= theta40*(1+2e-4)-2e-4
                # (a hair below theta40); count > 40 => boundary ambiguity.
                # Replaces the 6th max scan (1x) with a 2x tensor_scalar.
                thr40 = small.tile([128, 1], f32, tag="thr40")
                nc.vector.tensor_scalar(
                    out=thr40[:], in0=maxv[:, 39:40], scalar1=1.0 + 2e-4,
                    scalar2=-2e-4, op0=Alu.mult, op1=Alu.add)
                junk_c = work.tile([128, P], f32)
                nc.vector.tensor_scalar(
                    out=junk_c[:], in0=dsb[:], scalar1=thr40[:, 0:1],
                    scalar2=None, op0=Alu.is_ge, op1=Alu.add,
                    accum_out=sums[:, 3:4])

                nc.sync.dma_start(sums_o[bt, ro:ro + 128, :], sums[:])
                nc.sync.dma_start(maxv_o[bt, ro:ro + 128, :], maxv[:])

    nc.compile()
    return nc


def _hamilton(a, b):
    aw, ax, ay, az = a[..., 0], a[..., 1], a[..., 2], a[..., 3]
    bw, bx, by, bz = b[..., 0], b[..., 1], b[..., 2], b[..., 3]
    return np.stack([
        aw * bw - ax * bx - ay * by - az * bz,
        aw * bx + ax * bw + ay * bz - az * by,
        aw * by - ax * bz + ay * bw + az * bx,
        aw * bz + ax * by - ay * bx + az * bw,
    ], axis=-1).astype(np.float32)


def _host_prep(points):
    """numpy fp32 mirror of the reference preprocessing."""
    xyz = points[..., :3]
    cent = ((xyz.min(axis=2) + xyz.max(axis=2)) * np.float32(0.5))
    d = (xyz - cent[:, :, None, :]).astype(np.float32)
    n = np.sqrt((d * d).sum(-1, keepdims=True)).astype(np.float32)
    d = (d / np.maximum(n, np.float32(1e-12))).astype(np.float32)
    dot = np.clip(d[..., 1], np.float32(-1.0 + 1e-7), np.float32(1.0 - 1e-7))
    half = (np.arccos(dot) * np.float32(0.5)).astype(np.float32)
    axis = np.stack([d[..., 2], np.zeros_like(dot), -d[..., 0]], -1)
    an = np.sqrt((axis * axis).sum(-1, keepdims=True)).astype(np.float32)
    axis = (axis / np.maximum(an, np.float32(1e-12))).astype(np.float32)
    s = np.sin(half).astype(np.float32)
    bq = np.stack([np.cos(half).astype(np.float32), axis[..., 0] * s,
                   axis[..., 1] * s, axis[..., 2] * s], -1).astype(np.float32)
    conj = np.array([1, -1, -1, -1], np.float32)
    qf = _hamilton(bq[:, 1:], bq[:, :-1] * conj)
    qn = np.sqrt((qf * qf).sum(-1, keepdims=True)).astype(np.float32)
    qf = (qf / np.maximum(qn, np.float32(1e-12))).astype(np.float32)
    src = np.ascontiguousarray(xyz[:, :-1])          # (B,T,P,3)
    sq = (src * src).sum(-1).astype(np.float32)      # (B,T,P)
    return src, sq, qf


def _device_inputs(src, sq, qf):
    srcf = src.reshape(NBT, P, 3)
    sqf = sq.reshape(NBT, P)
    qff = qf.reshape(NBT, P, 4)

    aug_rhs = np.empty((NBT, AUGK, P), np.float32)
    aug_rhs[:, 0:3] = srcf.transpose(0, 2, 1)
    aug_rhs[:, 3] = -1.0
    aug_rhs[:, 4] = -sqf
    qf_rhs = np.ascontiguousarray(qff.transpose(0, 2, 1))

    lhs_full = np.empty((NBT, AUGK, P), np.float32)
    lhs_full[:, 0:3] = 2.0 * srcf.transpose(0, 2, 1)
    lhs_full[:, 3] = sqf
    lhs_full[:, 4] = 1.0

    in_maps = []
    for c in range(NCORES):
        sl = slice(c * CH, (c + 1) * CH)
        in_maps.append({
            "aug_lhsT": np.ascontiguousarray(lhs_full[:, :, sl]),
            "aug_rhs": aug_rhs,
            "qf_lhsT": np.ascontiguousarray(qf_rhs[:, :, sl]),
            "qf_rhs": qf_rhs,
        })
    return in_maps


def _run_device(in_maps, trace=False, trace_kwargs=None):
    global _PROG
    from concourse.bass_utils import run_bass_kernel_spmd
    if _PROG is None:
        _PROG = _build_program()
    kw = dict(trace_kwargs or {})
    res = run_bass_kernel_spmd(_PROG, in_maps, core_ids=list(range(NCORES)),
                               trace=trace, **kw)
    return res


def _host_post(results, src, sq, qf):
    # reassemble per-core outputs -> (NBT, P, .)
    sums = np.empty((NBT, P, 4), np.float32)
    maxv = np.empty((NBT, P, 40), np.float32)
    for c, r in enumerate(results):
        sl = slice(c * CH, (c + 1) * CH)
        sums[:, sl] = r["sums"].reshape(NBT, CH, 4)
        maxv[:, sl] = r["maxv"].reshape(NBT, CH, 40)

    # per-(b,t,p) topk means of geo; device g = arccos/2 -> geo-sum = 4*g-sum
    mean_tk = np.empty((3, NBT, P), np.float32)
    for i, k in enumerate(SCALES):
        mean_tk[i] = sums[:, :, i] * np.float32(4.0 / k)

    # flag rows where the rank boundary is ambiguous (ties / near-ties);
    # rank-40 ambiguity comes from the device-side margin count in sums[...,3]
    gap5 = maxv[:, :, 4] - maxv[:, :, 5]
    gap15 = maxv[:, :, 14] - maxv[:, :, 15]
    thr = np.maximum(1e-5 * np.abs(maxv[:, :, 4]), 2e-4).astype(np.float32)
    flags = (gap5 < thr) | (gap15 < thr) | (sums[:, :, 3] > 40.0)
    fbt, fp_ = np.nonzero(flags)
    if len(fbt) > 0:
        srcf = src.reshape(NBT, P, 3)
        sqf = sq.reshape(NBT, P)
        qff = qf.reshape(NBT, P, 4)
        for bt, p in zip(fbt, fp_):
            row = (2.0 * (srcf[bt] @ srcf[bt, p]) - sqf[bt]
                   - sqf[bt, p]).astype(np.float32)
            order = np.argsort(-row, kind="stable")[:max(SCALES)]
            dots = np.abs((qff[bt] @ qff[bt, p]).astype(np.float32)[order])
            dots = np.clip(dots, np.float32(0.0), np.float32(1.0 - 1e-7))
            geo = (2.0 * np.arccos(dots)).astype(np.float32)
            for i, k in enumerate(SCALES):
                mean_tk[i, bt, p] = geo[:k].mean(dtype=np.float32)

    # mean over transitions -> (3, B, P)
    mean_inc = mean_tk.reshape(3, B, T, P).mean(axis=2, dtype=np.float32)

    out = np.empty((B, 3, F * P), np.float32)
    for i in range(3):
        mi = mean_inc[i]
        flat = np.sort(mi.reshape(-1), kind="stable")
        scale = np.float32(max(flat[(flat.size - 1) // 2], np.float32(1e-6)))
        if mi.max() > 0:
            rig = np.exp(-mi / scale).astype(np.float32)
        else:
            rig = np.ones_like(mi)
        out[:, i, :] = np.broadcast_to(rig[:, None, :], (B, F, P)).reshape(B, F * P)
    return out


def kernel(points_4d, num_frames=None, _trace=False, _trace_kwargs=None):
    points = np.asarray(points_4d, dtype=np.float32)
    assert points.shape == (B, F, P, 4)
    src, sq, qf = _host_prep(points)
    in_maps = _device_inputs(src, sq, qf)
    res = _run_device(in_maps, trace=_trace, trace_kwargs=_trace_kwargs)
    out = _host_post(res.results, src, sq, qf)
    kernel._last_result = res
    return out



# revision 15
# speedup vs baseline: 2.0897x; 2.0897x over previous
"""Trainium2 Bass kernel for nn_BearingQCCFeatureMotion.

Architecture (B=2, F=8, P=2048, SCALES=(5,15,40)):

Device (8 cores, data-parallel over P; each core owns a 256-row slice of all
14 (b,t) transitions) computes ONLY the O(P^2) part — the pairwise-distance
matmul and an exact top-40 per row — on packed values that carry the column
index, so no gather and no per-element geodesic math is needed on device:

  - PE:   D[p,q] = 2<x_p,x_q> - |x_p|^2 - |x_q|^2 as a K=5 matmul (fp32).
  - Pool: pack   D'' = (bits(D) & 0xFFFFF800) | 0x80000000 | q
          (quantize the mantissa to 12 bits, force the sign so all values are
          negative, put the column index in the low 11 bits).  Descending
          order of D'' == ascending |D| with ties broken to the smaller
          index — exactly jax.lax.top_k's stable order on the quantized
          values, and every packed value is unique.
  - top-40 is hierarchical: each 512-wide quarter of the row contributes its
    top-16 (2x max8 + one exact inclusive erase: match_replace on DVE or
    mask = (cur is_ge v8)*1e18 ; cur' = cur + (-1)*mask on Pool), then the 64
    candidates are merged with 5x max8 + 4x match_replace at width 64.
    Rows where a quarter might hide top-40 members (its rank-16 value >=
    the merged rank-40 value) are flagged via 4 exported per-quarter
    rank-16 values and recomputed on host (~2% of rows).
  - ACT:  near-boundary count for the rank-40 tie flag:
          sum(Sign(D'' + thr)) with thr = |theta40|*(1+3*2^-12) + MARGIN_DEV.
  - DMA out: [128, 45] per chunk (40 packed values + count + 4 quarter-16s).

Host (numpy fp32, vectorized) does all O(P*k) work: unpack indices, gather
neighbor quaternion dots, geo = 2*arccos(|dot|), prefix means for k=5/15/40,
near-tie/ambiguity flags with exact recompute of flagged rows, mean over
transitions, lower-median scale, exp, broadcast to (B, 3, F*P).

Correctness containment: quantization is monotone, so device order can differ
from the reference fp32 order only where packed buckets merge (gap below one
11-bit bucket) or where device-vs-host matmul rounding can flip a boundary;
every such row is detected via bucket-gap / count / quarter-truncation flags
and recomputed exactly on host.
"""

import numpy as np

B, F, P = 2, 8, 2048
T = F - 1
NBT = B * T            # 14 independent (b, t) transitions
NCORES = 8
CH = P // NCORES       # 256 rows per core
SCALES = (5, 15, 40)
AUGK = 5               # [2x, 2y, 2z, sq, 1] . [x', y', z', -1, -sq']
KMAX = 40
NSEG = 4
SEGW = P // NSEG       # 512
OUTW = 45              # 40 maxv + count + 4 quarter rank-16s

IDX_BITS = 0x7FF                   # 11 low mantissa bits hold the column
PACK_AND = 0xFFFFF800              # quantize: clear the index bits
MAG_AND = 0x7FFFF800               # quantized magnitude (sign cleared)
BUCKET_REL = float(2.0 ** -12)     # one quantization bucket, relative
MARGIN_DEV = 6e-5                  # count-margin for rank-40 ambiguity (abs)
MARGIN_ABS = 3e-5                  # host near-tie margin for ranks 5/15 (abs)
COUNT_FLAG = 2 * 41 - 2048 - 1     # flag if sum(sign) >= this (count > 40)

_PROG = None


def _build_program(repeat=1):
    """Build + compile the per-core Bass/Tile program (same for all cores)."""
    from contextlib import ExitStack
    import concourse.tile as tile
    from concourse import bacc, mybir
    import concourse.bass as bass

    f32 = mybir.dt.float32
    u32 = mybir.dt.uint32
    Alu = mybir.AluOpType
    Act = mybir.ActivationFunctionType

    nc = bacc.Bacc("TRN2", target_bir_lowering=False, debug=False)

    aug_lhsT = nc.dram_tensor("aug_lhsT", [NBT, AUGK, CH], f32,
                              kind="ExternalInput").ap()
    aug_rhs = nc.dram_tensor("aug_rhs", [NBT, AUGK, P], f32,
                             kind="ExternalInput").ap()
    iota_i = nc.dram_tensor("iota", [128, P], u32, kind="ExternalInput").ap()
    pk_o = nc.dram_tensor("pk", [NBT, CH, OUTW], f32,
                          kind="ExternalOutput").ap()

    with tile.TileContext(nc) as tc, ExitStack() as ctx:
        const = ctx.enter_context(tc.tile_pool(name="const", bufs=1))
        iota = const.tile([128, P], u32)
        nc.sync.dma_start(iota[:], iota_i[:])
        c_pack = const.tile([128, 1], u32)
        nc.vector.memset(c_pack[:], PACK_AND)
        c_mag = const.tile([128, 1], u32)
        nc.vector.memset(c_mag[:], MAG_AND)

        tabs = ctx.enter_context(tc.tile_pool(name="tabs", bufs=2))
        psum_d = ctx.enter_context(tc.tile_pool(name="psd", bufs=2,
                                                space="PSUM"))
        work = ctx.enter_context(tc.tile_pool(name="work", bufs=2))
        segp = ctx.enter_context(tc.tile_pool(name="segp", bufs=2))
        small = ctx.enter_context(tc.tile_pool(name="small", bufs=3))
        tiny = ctx.enter_context(tc.tile_pool(name="tiny", bufs=3))

        for bt in [i % NBT for i in range(NBT * repeat)]:
            a_rhs = tabs.tile([AUGK, P], f32, tag="a_rhs")
            nc.sync.dma_start(a_rhs[:], aug_rhs[bt])
            a_lhs = tabs.tile([AUGK, CH], f32, tag="a_lhs")
            nc.sync.dma_start(a_lhs[:], aug_lhsT[bt])

            for ch in range(CH // 128):
                ro = ch * 128

                psd = psum_d.tile([128, P], f32)
                for j in range(4):
                    nc.tensor.matmul(
                        psd[:, bass.ts(j, 512)],
                        lhsT=a_lhs[:, ro:ro + 128],
                        rhs=a_rhs[:, bass.ts(j, 512)],
                        start=True, stop=True)

                # pack straight from PSUM on DVE (stt is DVE-only; GPSIMD
                # can't read PSUM): quantize mantissa, force sign, OR index
                dpk = work.tile([128, P], f32, tag="dpk")
                nc.vector.scalar_tensor_tensor(
                    out=dpk[:].bitcast(u32), in0=psd[:].bitcast(u32),
                    scalar=c_pack[:, 0:1], in1=iota[:],
                    op0=Alu.bitwise_and, op1=Alu.bitwise_or)

                cand = small.tile([128, 64], f32, tag="cand")
                out = small.tile([128, OUTW], f32, tag="out")

                # quarter round 1: top-8 of each 512-wide quarter
                for s in range(NSEG):
                    nc.vector.max(cand[:, 16 * s:16 * s + 8],
                                  dpk[:, SEGW * s:SEGW * (s + 1)])
                # exact inclusive erase of each quarter's top-8
                wks = {}
                for s in range(NSEG):
                    sl = slice(SEGW * s, SEGW * (s + 1))
                    wks[s] = segp.tile([128, SEGW], f32, tag=f"wks{s}",
                                       name=f"wks{s}")
                    msk = segp.tile([128, SEGW], f32, tag=f"msk{s}",
                                    name=f"msk{s}")
                    nc.gpsimd.tensor_scalar(
                        out=msk[:], in0=dpk[:, sl],
                        scalar1=cand[:, 16 * s + 7:16 * s + 8],
                        scalar2=1e18, op0=Alu.is_ge, op1=Alu.mult)
                    nc.gpsimd.tensor_tensor(
                        out=wks[s][:], in0=dpk[:, sl], in1=msk[:],
                        op=Alu.subtract)
                # quarter round 2: ranks 9-16 of each quarter
                for s in range(NSEG):
                    nc.vector.max(cand[:, 16 * s + 8:16 * s + 16], wks[s][:])
                # export quarter rank-16s for the truncation flag (before the
                # merge erases them from cand)
                for s in range(NSEG):
                    nc.vector.tensor_copy(out[:, 41 + s:42 + s],
                                          cand[:, 16 * s + 15:16 * s + 16])

                # merge 64 candidates -> exact top-40
                cc = cand
                for r in range(5):
                    nc.vector.max(out[:, 8 * r:8 * r + 8], cc[:])
                    if r == 4:
                        break
                    nc2 = small.tile([128, 64], f32, tag=f"cc{r % 2}",
                                     name=f"cc{r}")
                    nc.vector.match_replace(nc2[:], out[:, 8 * r:8 * r + 8],
                                            cc[:], -1e18)
                    cc = nc2

                # rank-40 ambiguity count: #{|D| <= |theta40|*(1+3b) + m}
                mag = tiny.tile([128, 1], u32, tag="mag")
                nc.vector.scalar_tensor_tensor(
                    out=mag[:], in0=out[:, 39:40].bitcast(u32),
                    scalar=c_mag[:, 0:1], in1=c_mag[:, 0:1],
                    op0=Alu.bitwise_and, op1=Alu.bitwise_and)
                thr = tiny.tile([128, 1], f32, tag="thr")
                nc.vector.tensor_scalar(
                    out=thr[:], in0=mag[:].bitcast(f32),
                    scalar1=1.0 + 3.0 * BUCKET_REL, scalar2=MARGIN_DEV,
                    op0=Alu.mult, op1=Alu.add)
                junk = work.tile([128, P], f32, tag="junk")
                nc.scalar.activation(junk[:], dpk[:], Act.Sign,
                                     bias=thr[:, 0:1], scale=1.0,
                                     accum_out=out[:, 40:41])

                nc.sync.dma_start(pk_o[bt, ro:ro + 128, :], out[:])

    nc.compile()
    return nc


def _hamilton(a, b):
    aw, ax, ay, az = a[..., 0], a[..., 1], a[..., 2], a[..., 3]
    bw, bx, by, bz = b[..., 0], b[..., 1], b[..., 2], b[..., 3]
    return np.stack([
        aw * bw - ax * bx - ay * by - az * bz,
        aw * bx + ax * bw + ay * bz - az * by,
        aw * by - ax * bz + ay * bw + az * bx,
        aw * bz + ax * by - ay * bx + az * bw,
    ], axis=-1).astype(np.float32)


def _host_prep(points):
    """numpy fp32 mirror of the reference preprocessing."""
    xyz = points[..., :3]
    cent = ((xyz.min(axis=2) + xyz.max(axis=2)) * np.float32(0.5))
    d = (xyz - cent[:, :, None, :]).astype(np.float32)
    n = np.sqrt((d * d).sum(-1, keepdims=True)).astype(np.float32)
    d = (d / np.maximum(n, np.float32(1e-12))).astype(np.float32)
    dot = np.clip(d[..., 1], np.float32(-1.0 + 1e-7), np.float32(1.0 - 1e-7))
    half = (np.arccos(dot) * np.float32(0.5)).astype(np.float32)
    axis = np.stack([d[..., 2], np.zeros_like(dot), -d[..., 0]], -1)
    an = np.sqrt((axis * axis).sum(-1, keepdims=True)).astype(np.float32)
    axis = (axis / np.maximum(an, np.float32(1e-12))).astype(np.float32)
    s = np.sin(half).astype(np.float32)
    bq = np.stack([np.cos(half).astype(np.float32), axis[..., 0] * s,
                   axis[..., 1] * s, axis[..., 2] * s], -1).astype(np.float32)
    conj = np.array([1, -1, -1, -1], np.float32)
    qf = _hamilton(bq[:, 1:], bq[:, :-1] * conj)
    qn = np.sqrt((qf * qf).sum(-1, keepdims=True)).astype(np.float32)
    qf = (qf / np.maximum(qn, np.float32(1e-12))).astype(np.float32)
    src = np.ascontiguousarray(xyz[:, :-1])          # (B,T,P,3)
    sq = (src * src).sum(-1).astype(np.float32)      # (B,T,P)
    return src, sq, qf


def _device_inputs(src, sq, qf):
    srcf = src.reshape(NBT, P, 3)
    sqf = sq.reshape(NBT, P)

    aug_rhs = np.empty((NBT, AUGK, P), np.float32)
    aug_rhs[:, 0:3] = srcf.transpose(0, 2, 1)
    aug_rhs[:, 3] = -1.0
    aug_rhs[:, 4] = -sqf

    lhs_full = np.empty((NBT, AUGK, P), np.float32)
    lhs_full[:, 0:3] = 2.0 * srcf.transpose(0, 2, 1)
    lhs_full[:, 3] = sqf
    lhs_full[:, 4] = 1.0

    iota = np.broadcast_to(
        np.arange(P, dtype=np.uint32) | np.uint32(0x80000000),
        (128, P)).copy()

    in_maps = []
    for c in range(NCORES):
        sl = slice(c * CH, (c + 1) * CH)
        in_maps.append({
            "aug_lhsT": np.ascontiguousarray(lhs_full[:, :, sl]),
            "aug_rhs": aug_rhs,
            "iota": iota,
        })
    return in_maps


def _run_device(in_maps, trace=False, trace_kwargs=None):
    global _PROG
    from concourse.bass_utils import run_bass_kernel_spmd
    if _PROG is None:
        _PROG = _build_program()
    kw = dict(trace_kwargs or {})
    res = run_bass_kernel_spmd(_PROG, in_maps, core_ids=list(range(NCORES)),
                               trace=trace, **kw)
    return res


def _geo_from_idx(qff, idx):
    """geo = 2*arccos(clip(|<q_p, q_idx>|)) for idx [..., k] (fp32)."""
    nbr = np.take_along_axis(
        qff[:, :, None, :], idx[..., None].astype(np.int64), axis=1)
    dots = np.abs((qff[:, :, None, :] * nbr).sum(-1, dtype=np.float32))
    dots = np.clip(dots, np.float32(0.0), np.float32(1.0 - 1e-7))
    return (2.0 * np.arccos(dots)).astype(np.float32)


def _host_post(results, src, sq, qf):
    # reassemble per-core outputs -> (NBT, P, OUTW)
    pk = np.empty((NBT, P, OUTW), np.float32)
    for c, r in enumerate(results):
        sl = slice(c * CH, (c + 1) * CH)
        pk[:, sl] = r["pk"].reshape(NBT, CH, OUTW)

    bits = np.ascontiguousarray(pk[:, :, :KMAX]).view(np.uint32)
    idx = (bits & np.uint32(IDX_BITS)).astype(np.int64)    # (NBT, P, 40)
    mq = (bits & np.uint32(MAG_AND)).view(np.float32)      # |D| quantized, asc
    cnt = pk[:, :, 40]

    # ---- flags ----
    gap5 = mq[:, :, 5] - mq[:, :, 4]
    gap15 = mq[:, :, 15] - mq[:, :, 14]
    thr5 = np.maximum(3.0 * BUCKET_REL * mq[:, :, 5], MARGIN_ABS)
    thr15 = np.maximum(3.0 * BUCKET_REL * mq[:, :, 15], MARGIN_ABS)
    flags = (gap5 <= thr5) | (gap15 <= thr15) | (cnt >= COUNT_FLAG)
    # quarter truncation: a quarter's rank-16 reached the merged rank-40
    flags |= pk[:, :, 41:45].max(axis=-1) >= pk[:, :, 39]
    # safety: reflection ambiguity near zero / denormal flush losing the index
    flags |= mq[:, :, 4] <= np.float32(4.0 * MARGIN_ABS)
    flags |= bits[:, :, 0] == np.uint32(0x80000000)

    qff = qf.reshape(NBT, P, 4)
    geo = _geo_from_idx(qff, idx)                          # (NBT, P, 40)

    mean_tk = np.empty((3, NBT, P), np.float32)
    for i, k in enumerate(SCALES):
        mean_tk[i] = geo[:, :, :k].mean(axis=-1, dtype=np.float32)

    # ---- exact recompute of flagged rows (vectorized per transition) ----
    srcf = src.reshape(NBT, P, 3)
    sqf = sq.reshape(NBT, P)
    fbt, fp_ = np.nonzero(flags)
    if len(fbt) > 0:
        for bt in np.unique(fbt):
            rows = fp_[fbt == bt]
            rowD = (2.0 * (srcf[bt, rows] @ srcf[bt].T)
                    - sqf[bt, rows][:, None] - sqf[bt][None, :]).astype(np.float32)
            order = np.argsort(-rowD, axis=-1, kind="stable")[:, :KMAX]
            nbr = qff[bt][order]                            # (R, 40, 4)
            dots = np.abs((qff[bt, rows][:, None, :] * nbr)
                          .sum(-1, dtype=np.float32))
            dots = np.clip(dots, np.float32(0.0), np.float32(1.0 - 1e-7))
            g = (2.0 * np.arccos(dots)).astype(np.float32)
            for i, k in enumerate(SCALES):
                mean_tk[i, bt, rows] = g[:, :k].mean(axis=-1, dtype=np.float32)

    # mean over transitions -> (3, B, P)
    mean_inc = mean_tk.reshape(3, B, T, P).mean(axis=2, dtype=np.float32)

    out = np.empty((B, 3, F * P), np.float32)
    for i in range(3):
        mi = mean_inc[i]
        flat = np.sort(mi.reshape(-1), kind="stable")
        scale = np.float32(max(flat[(flat.size - 1) // 2], np.float32(1e-6)))
        if mi.max() > 0:
            rig = np.exp(-mi / scale).astype(np.float32)
        else:
            rig = np.ones_like(mi)
        out[:, i, :] = np.broadcast_to(rig[:, None, :], (B, F, P)).reshape(B, F * P)
    kernel._n_flagged = int(flags.sum())
    return out


def kernel(points_4d, num_frames=None, _trace=False, _trace_kwargs=None):
    points = np.asarray(points_4d, dtype=np.float32)
    assert points.shape == (B, F, P, 4)
    src, sq, qf = _host_prep(points)
    in_maps = _device_inputs(src, sq, qf)
    res = _run_device(in_maps, trace=_trace, trace_kwargs=_trace_kwargs)
    out = _host_post(res.results, src, sq, qf)
    kernel._last_result = res
    return out
